# revision 2
# baseline (speedup 1.0000x reference)
"""Trainium2 Bass kernel for AdaptedEmbedding (embedding gather + LoRA).

out[b,s,:] = emb_weight[input[b,s], :] + (lora_A[:, input[b,s]].T @ lora_B.T) * (alpha/r)

Strategy (data-parallel over tokens, no collectives):
  Host:
    - Compact the vocab: only the unique tokens of this batch (U <= 16384)
      are shipped, as W[u] = [emb_weight[uniq[u]] | lora_A[:, uniq[u]]].
    - Token ids -> ranks into the compacted table (int32).
    - Fold the LoRA scaling (alpha/r = 2.0) into B^T.
    - Shard the 16384 tokens contiguously: 2048 tokens per core.
  Device (per core):
    - For each 128-token chunk: indirect-DMA gather of W rows into SBUF,
      transpose the 16-wide A-part via TensorE, matmul with B^T (16x1024),
      vector-add onto the gathered embedding rows, DMA the result out.
  Host: concatenate per-core outputs -> (4, 4096, 1024).
"""

import numpy as np

B, S = 4, 4096
DIM = 1024
R = 16
SCALING = 2.0
N_CORES = 8
TOK_PER_CORE = (B * S) // N_CORES  # 2048
P = 128
N_CHUNKS = TOK_PER_CORE // P  # 16
WROW = DIM + R  # 1040


def _build_graph(u_rows: int):
    import concourse.bacc as bacc
    import concourse.bass as bass
    import concourse.mybir as mybir
    import concourse.tile as tile
    from concourse.masks import make_identity

    f32 = mybir.dt.float32

    nc = bacc.Bacc("TRN2", target_bir_lowering=False)

    idx = nc.declare_dram_parameter("idx", [P, N_CHUNKS], mybir.dt.int32, isOutput=False)
    w = nc.declare_dram_parameter("w", [u_rows, WROW], f32, isOutput=False)
    bt = nc.declare_dram_parameter("bt", [R, DIM], f32, isOutput=False)
    out = nc.declare_dram_parameter("out", [TOK_PER_CORE, DIM], f32, isOutput=True)
    out_r = out[:].rearrange("(c p) d -> p c d", p=P)  # [128, 16, 1024]

    with tile.TileContext(nc) as tc:
        with (
            tc.tile_pool(name="persist", bufs=1) as pers,
            tc.tile_pool(name="sbuf", bufs=3) as sb,
            tc.tile_pool(name="psum", bufs=2, space="PSUM") as ps,
        ):
            identity = pers.tile([P, P], dtype=f32)
            make_identity(nc, identity[:])
            bt_sb = pers.tile([R, DIM], dtype=f32)
            nc.sync.dma_start(out=bt_sb[:], in_=bt[:])
            idx_sb = pers.tile([P, N_CHUNKS], dtype=mybir.dt.int32)
            nc.sync.dma_start(out=idx_sb[:], in_=idx[:])

            for c in range(N_CHUNKS):
                g = sb.tile([P, WROW], dtype=f32, tag="g")
                nc.gpsimd.indirect_dma_start(
                    out=g[:],
                    out_offset=None,
                    in_=w[:],
                    in_offset=bass.IndirectOffsetOnAxis(ap=idx_sb[:, c : c + 1], axis=0),
                )
                # a_t = (g[:, 1024:1040])^T : [16, 128]
                a_t_ps = ps.tile([R, P], dtype=f32, tag="a_t_ps")
                nc.tensor.transpose(out=a_t_ps[:], in_=g[:, DIM:WROW], identity=identity[:])
                a_t = sb.tile([R, P], dtype=f32, tag="a_t")
                nc.vector.tensor_copy(out=a_t[:], in_=a_t_ps[:])
                # lora = a_t.T @ bt : [128, 1024], in two 512-wide matmuls
                lora_ps = ps.tile([P, DIM], dtype=f32, tag="lora_ps")
                for h in range(2):
                    cs = slice(h * 512, (h + 1) * 512)
                    nc.tensor.matmul(
                        out=lora_ps[:, cs], lhsT=a_t[:], rhs=bt_sb[:, cs],
                        start=True, stop=True,
                    )
                    nc.vector.tensor_add(out=g[:, cs], in0=g[:, cs], in1=lora_ps[:, cs])
                nc.sync.dma_start(out=out_r[:, c, :], in_=g[:, :DIM])

    nc.finalize()
    return nc


def kernel(input, emb_weight, lora_A, lora_B):
    from concourse.bass_utils import run_bass_kernel_spmd

    ids = np.asarray(input).astype(np.int64).reshape(-1)  # (16384,)
    emb_weight = np.asarray(emb_weight, dtype=np.float32)
    lora_A = np.asarray(lora_A, dtype=np.float32)
    lora_B = np.asarray(lora_B, dtype=np.float32)

    uniq, inv = np.unique(ids, return_inverse=True)
    u_rows = len(uniq)
    w_host = np.empty((u_rows, WROW), dtype=np.float32)
    w_host[:, :DIM] = emb_weight[uniq]
    w_host[:, DIM:] = lora_A[:, uniq].T
    bt_host = np.ascontiguousarray((lora_B * SCALING).T)  # (16, 1024)

    inv32 = inv.astype(np.int32)
    in_maps = []
    for c in range(N_CORES):
        sl = inv32[c * TOK_PER_CORE : (c + 1) * TOK_PER_CORE]
        # SBUF layout: [partition p, chunk c] holds token c*128+p
        idx_core = np.ascontiguousarray(sl.reshape(N_CHUNKS, P).T)
        in_maps.append({"idx": idx_core, "w": w_host, "bt": bt_host})

    nc = _build_graph(u_rows)
    res = run_bass_kernel_spmd(nc, in_maps, list(range(N_CORES)))
    out = np.concatenate([res.results[i]["out"] for i in range(N_CORES)], axis=0)
    return out.reshape(B, S, DIM)


# revision 4
# speedup vs baseline: 1.3624x; 1.3624x over previous
"""Trainium2 Bass kernel for AdaptedEmbedding (embedding gather + LoRA).

out[b,s,:] = emb_weight[input[b,s], :] + (lora_A[:, input[b,s]].T @ lora_B.T) * (alpha/r)

Strategy (data-parallel over tokens, no collectives):
  Host:
    - Compact the vocab: only the unique tokens of this batch (U <= 16384)
      are shipped; all device indices then fit in int16 for dma_gather.
    - emb_small[U, 1024] f32; bt[16, 1024] bf16 = (lora_B * scaling)^T.
    - Per core: the (tiny, 64KB) per-token A^T slice is shipped dense and
      pre-transposed as at[16, 2048] bf16 -- the heavy gather (embedding
      rows, 8MB/core) runs on device.
    - Shard the 16384 tokens contiguously: 2048 tokens per core.
  Device (per core):
    - 4 dma_gathers of 512 tokens each pull embedding rows (pipelined).
    - Per 128-token chunk: 2 bf16 matmuls (K=16) with bt, f32 vector-add
      onto the gathered rows, group-level DMA writeback.
  Host: concatenate per-core outputs -> (4, 4096, 1024).
"""

import numpy as np

B, S = 4, 4096
DIM = 1024
R = 16
SCALING = 2.0
N_CORES = 8
TOK_PER_CORE = (B * S) // N_CORES  # 2048
P = 128
N_GROUPS = 4
GROUP_TOK = TOK_PER_CORE // N_GROUPS  # 512
CHUNKS_PER_GROUP = GROUP_TOK // P  # 4
GROUP_IDX_COLS = GROUP_TOK // 16  # 32


def _build_graph(u_rows: int):
    import concourse.bacc as bacc
    import concourse.mybir as mybir
    import concourse.tile as tile

    f32 = mybir.dt.float32
    bf16 = mybir.dt.bfloat16
    i16 = mybir.dt.int16

    nc = bacc.Bacc("TRN2", target_bir_lowering=False)

    idx = nc.declare_dram_parameter(
        "idx", [N_GROUPS, P, GROUP_IDX_COLS], i16, isOutput=False
    )
    emb = nc.declare_dram_parameter("emb", [u_rows, DIM], f32, isOutput=False)
    at = nc.declare_dram_parameter("at", [R, TOK_PER_CORE], bf16, isOutput=False)
    bt = nc.declare_dram_parameter("bt", [R, DIM], bf16, isOutput=False)
    out = nc.declare_dram_parameter("out", [TOK_PER_CORE, DIM], f32, isOutput=True)
    # row k*512 + c*128 + p  ->  [k, p, c, d]
    out_r = out[:].rearrange("(k c p) d -> k p c d", p=P, c=CHUNKS_PER_GROUP)

    with tile.TileContext(nc) as tc:
        with (
            tc.tile_pool(name="persist", bufs=1) as pers,
            tc.tile_pool(name="sbuf", bufs=2) as sb,
            tc.tile_pool(name="psum", bufs=2, space="PSUM") as ps,
        ):
            bt_sb = pers.tile([R, DIM], dtype=bf16)
            nc.sync.dma_start(out=bt_sb[:], in_=bt[:])
            a_t = pers.tile([R, TOK_PER_CORE], dtype=bf16)
            nc.sync.dma_start(out=a_t[:], in_=at[:])
            idx_sb = []
            for k in range(N_GROUPS):
                t = pers.tile([P, GROUP_IDX_COLS], dtype=i16, tag=f"idx{k}")
                nc.sync.dma_start(out=t[:], in_=idx[k])
                idx_sb.append(t)

            for k in range(N_GROUPS):
                g = sb.tile([P, CHUNKS_PER_GROUP, DIM], dtype=f32, tag="g")
                nc.gpsimd.dma_gather(
                    g[:], emb[:], idx_sb[k][:], GROUP_TOK, GROUP_TOK, DIM,
                )
                for c in range(CHUNKS_PER_GROUP):
                    tok0 = k * GROUP_TOK + c * P
                    lora_ps = ps.tile([P, DIM], dtype=f32, tag="lora_ps")
                    for h in range(2):
                        cs = slice(h * 512, (h + 1) * 512)
                        nc.tensor.matmul(
                            out=lora_ps[:, cs],
                            lhsT=a_t[:, tok0 : tok0 + P],
                            rhs=bt_sb[:, cs],
                            start=True, stop=True,
                        )
                        nc.vector.tensor_add(
                            out=g[:, c, cs], in0=g[:, c, cs], in1=lora_ps[:, cs]
                        )
                nc.sync.dma_start(out=out_r[k], in_=g[:])

    nc.finalize()
    return nc


def kernel(input, emb_weight, lora_A, lora_B):
    import ml_dtypes
    from concourse.bass_utils import run_bass_kernel_spmd

    ids = np.asarray(input).astype(np.int64).reshape(-1)  # (16384,)
    emb_weight = np.asarray(emb_weight, dtype=np.float32)
    lora_A = np.asarray(lora_A, dtype=np.float32)
    lora_B = np.asarray(lora_B, dtype=np.float32)

    uniq, inv = np.unique(ids, return_inverse=True)
    u_rows = len(uniq)
    emb_small = np.ascontiguousarray(emb_weight[uniq])
    bt_host = np.ascontiguousarray((lora_B * SCALING).T).astype(ml_dtypes.bfloat16)

    inv16 = inv.astype(np.int16)
    in_maps = []
    for c in range(N_CORES):
        sl = inv16[c * TOK_PER_CORE : (c + 1) * TOK_PER_CORE]
        # wrapped idx layout per 512-token group: token j at [j % 16, j // 16],
        # replicated over the 8 GPSIMD core partition-groups
        idx_core = np.stack(
            [
                np.tile(sl[k * GROUP_TOK : (k + 1) * GROUP_TOK].reshape(-1, 16).T, (8, 1))
                for k in range(N_GROUPS)
            ]
        )
        at_core = np.ascontiguousarray(
            lora_A[:, ids[c * TOK_PER_CORE : (c + 1) * TOK_PER_CORE]]
        ).astype(ml_dtypes.bfloat16)
        in_maps.append({"idx": idx_core, "emb": emb_small, "at": at_core, "bt": bt_host})

    nc = _build_graph(u_rows)
    res = run_bass_kernel_spmd(nc, in_maps, list(range(N_CORES)))
    out = np.concatenate([res.results[i]["out"] for i in range(N_CORES)], axis=0)
    return out.reshape(B, S, DIM)


# revision 5
# speedup vs baseline: 1.6104x; 1.1820x over previous
"""Trainium2 Bass kernel for AdaptedEmbedding (embedding gather + LoRA).

out[b,s,:] = emb_weight[input[b,s], :] + (lora_A[:, input[b,s]].T @ lora_B.T) * (alpha/r)

Strategy (data-parallel over tokens, no collectives):
  Host:
    - Compact the vocab: only the unique tokens of this batch (U <= 16384)
      are shipped; all device indices then fit in int16 for dma_gather.
    - emb_small[U, 1024] f32; bt[16, 1024] bf16 = (lora_B * scaling)^T.
    - Per core: the (tiny, 64KB) per-token A^T slice is shipped dense and
      pre-transposed as at[16, 2048] bf16 -- the heavy gather (embedding
      rows, 8MB/core) runs on device.
    - Shard the 16384 tokens contiguously: 2048 tokens per core.
  Device (per core):
    - 8 dma_gathers of 256 tokens each pull embedding rows (pipelined,
      4 buffers deep).
    - Per 128-token chunk: 2 bf16 matmuls (K=16) with bt into PSUM, one
      1024-wide f32 vector-add (gathered + lora) into a staging tile,
      per-chunk DMA writeback.
  Host: concatenate per-core outputs -> (4, 4096, 1024).
"""

import numpy as np

B, S = 4, 4096
DIM = 1024
R = 16
SCALING = 2.0
N_CORES = 8
TOK_PER_CORE = (B * S) // N_CORES  # 2048
P = 128
N_GROUPS = 8
GROUP_TOK = TOK_PER_CORE // N_GROUPS  # 256
CHUNKS_PER_GROUP = GROUP_TOK // P  # 2
GROUP_IDX_COLS = GROUP_TOK // 16  # 16


def _build_graph(u_rows: int):
    import concourse.bacc as bacc
    import concourse.mybir as mybir
    import concourse.tile as tile

    f32 = mybir.dt.float32
    bf16 = mybir.dt.bfloat16
    i16 = mybir.dt.int16

    nc = bacc.Bacc("TRN2", target_bir_lowering=False)

    idx = nc.declare_dram_parameter(
        "idx", [N_GROUPS, P, GROUP_IDX_COLS], i16, isOutput=False
    )
    emb = nc.declare_dram_parameter("emb", [u_rows, DIM], f32, isOutput=False)
    at = nc.declare_dram_parameter("at", [R, TOK_PER_CORE], bf16, isOutput=False)
    bt = nc.declare_dram_parameter("bt", [R, DIM], bf16, isOutput=False)
    out = nc.declare_dram_parameter("out", [TOK_PER_CORE, DIM], f32, isOutput=True)

    with tile.TileContext(nc) as tc:
        with (
            tc.tile_pool(name="persist", bufs=1) as pers,
            tc.tile_pool(name="sbuf", bufs=4) as sb,
            tc.tile_pool(name="outp", bufs=3) as op,
            tc.tile_pool(name="psum", bufs=3, space="PSUM") as ps,
        ):
            idx_sb = []
            for k in range(N_GROUPS):
                t = pers.tile([P, GROUP_IDX_COLS], dtype=i16, tag=f"idx{k}")
                nc.sync.dma_start(out=t[:], in_=idx[k])
                idx_sb.append(t)
            bt_sb = pers.tile([R, DIM], dtype=bf16)
            nc.sync.dma_start(out=bt_sb[:], in_=bt[:])
            a_t = pers.tile([R, TOK_PER_CORE], dtype=bf16)
            nc.sync.dma_start(out=a_t[:], in_=at[:])

            for k in range(N_GROUPS):
                g = sb.tile([P, CHUNKS_PER_GROUP, DIM], dtype=f32, tag="g")
                nc.gpsimd.dma_gather(
                    g[:], emb[:], idx_sb[k][:], GROUP_TOK, GROUP_TOK, DIM,
                )
                for c in range(CHUNKS_PER_GROUP):
                    tok0 = k * GROUP_TOK + c * P
                    lora_ps = ps.tile([P, DIM], dtype=f32, tag="lora_ps")
                    for h in range(2):
                        cs = slice(h * 512, (h + 1) * 512)
                        nc.tensor.matmul(
                            out=lora_ps[:, cs],
                            lhsT=a_t[:, tok0 : tok0 + P],
                            rhs=bt_sb[:, cs],
                            start=True, stop=True,
                        )
                    o = op.tile([P, DIM], dtype=f32, tag="o")
                    nc.vector.tensor_add(out=o[:], in0=g[:, c, :], in1=lora_ps[:])
                    nc.sync.dma_start(out=out[tok0 : tok0 + P, :], in_=o[:])

    nc.finalize()
    return nc


def kernel(input, emb_weight, lora_A, lora_B):
    import ml_dtypes
    from concourse.bass_utils import run_bass_kernel_spmd

    ids = np.asarray(input).astype(np.int64).reshape(-1)  # (16384,)
    emb_weight = np.asarray(emb_weight, dtype=np.float32)
    lora_A = np.asarray(lora_A, dtype=np.float32)
    lora_B = np.asarray(lora_B, dtype=np.float32)

    uniq, inv = np.unique(ids, return_inverse=True)
    u_rows = len(uniq)
    emb_small = np.ascontiguousarray(emb_weight[uniq])
    bt_host = np.ascontiguousarray((lora_B * SCALING).T).astype(ml_dtypes.bfloat16)

    inv16 = inv.astype(np.int16)
    in_maps = []
    for c in range(N_CORES):
        sl = inv16[c * TOK_PER_CORE : (c + 1) * TOK_PER_CORE]
        # wrapped idx layout per group: token j at [j % 16, j // 16],
        # replicated over the 8 GPSIMD core partition-groups
        idx_core = np.stack(
            [
                np.tile(sl[k * GROUP_TOK : (k + 1) * GROUP_TOK].reshape(-1, 16).T, (8, 1))
                for k in range(N_GROUPS)
            ]
        )
        at_core = np.ascontiguousarray(
            lora_A[:, ids[c * TOK_PER_CORE : (c + 1) * TOK_PER_CORE]]
        ).astype(ml_dtypes.bfloat16)
        in_maps.append({"idx": idx_core, "emb": emb_small, "at": at_core, "bt": bt_host})

    nc = _build_graph(u_rows)
    res = run_bass_kernel_spmd(nc, in_maps, list(range(N_CORES)))
    out = np.concatenate([res.results[i]["out"] for i in range(N_CORES)], axis=0)
    return out.reshape(B, S, DIM)


# revision 10
# speedup vs baseline: 1.7172x; 1.0663x over previous
"""Trainium2 Bass kernel for AdaptedEmbedding (embedding gather + LoRA).

out[b,s,:] = emb_weight[input[b,s], :] + (lora_A[:, input[b,s]].T @ lora_B.T) * (alpha/r)

Strategy (data-parallel over tokens, no collectives):
  Host:
    - Compact the vocab: only the unique tokens of this batch (U <= 16384)
      are shipped; all device indices then fit in int16 for dma_gather.
    - emb_small[U, 1024] f32; bt[16, 1024] bf16 = (lora_B * scaling)^T.
    - Per core: the (tiny, 64KB) per-token A^T slice is shipped dense and
      pre-transposed as at[16, 2048] bf16 -- the heavy gather (embedding
      rows, 8MB/core) runs on device.
    - Shard the 16384 tokens contiguously: 2048 tokens per core.
  Device (per core):
    - 8 dma_gathers of 256 tokens each pull embedding rows (pipelined,
      4 buffers deep).
    - Per 128-token chunk: 2 bf16 matmuls (K=16) with bt into PSUM, one
      1024-wide f32 vector-add (gathered + lora) into a staging tile,
      per-chunk DMA writeback.
  Host: concatenate per-core outputs -> (4, 4096, 1024).
"""

import numpy as np

B, S = 4, 4096
DIM = 1024
R = 16
SCALING = 2.0
N_CORES = 8
TOK_PER_CORE = (B * S) // N_CORES  # 2048
P = 128
N_GROUPS = 8
GROUP_TOK = TOK_PER_CORE // N_GROUPS  # 256
CHUNKS_PER_GROUP = GROUP_TOK // P  # 2
GROUP_IDX_COLS = GROUP_TOK // 16  # 16
EMB_BF16 = True  # bf16 embedding table: halves gather traffic, rel err ~1e-3


def _build_graph(u_rows: int):
    import concourse.bacc as bacc
    import concourse.mybir as mybir
    import concourse.tile as tile

    f32 = mybir.dt.float32
    bf16 = mybir.dt.bfloat16
    i16 = mybir.dt.int16

    nc = bacc.Bacc("TRN2", target_bir_lowering=False)

    emb_dt = bf16 if EMB_BF16 else f32
    idx = nc.declare_dram_parameter(
        "idx", [N_GROUPS, P, GROUP_IDX_COLS], i16, isOutput=False
    )
    emb = nc.declare_dram_parameter("emb", [u_rows, DIM], emb_dt, isOutput=False)
    at = nc.declare_dram_parameter("at", [R, TOK_PER_CORE], bf16, isOutput=False)
    bt = nc.declare_dram_parameter("bt", [R, DIM], bf16, isOutput=False)
    out = nc.declare_dram_parameter("out", [TOK_PER_CORE, DIM], f32, isOutput=True)

    with tile.TileContext(nc) as tc:
        with (
            tc.tile_pool(name="persist", bufs=1) as pers,
            tc.tile_pool(name="sbuf", bufs=6) as sb,
            tc.tile_pool(name="outp", bufs=4) as op,
            tc.tile_pool(name="psum", bufs=3, space="PSUM") as ps,
        ):
            idx_sb = []
            for k in range(N_GROUPS):
                t = pers.tile([P, GROUP_IDX_COLS], dtype=i16, tag=f"idx{k}")
                nc.sync.dma_start(out=t[:], in_=idx[k])
                idx_sb.append(t)
            bt_sb = pers.tile([R, DIM], dtype=bf16)
            nc.sync.dma_start(out=bt_sb[:], in_=bt[:])
            a_t = pers.tile([R, TOK_PER_CORE], dtype=bf16)
            nc.sync.dma_start(out=a_t[:], in_=at[:])

            for k in range(N_GROUPS):
                g = sb.tile([P, CHUNKS_PER_GROUP, DIM], dtype=emb_dt, tag="g")
                nc.gpsimd.dma_gather(
                    g[:], emb[:], idx_sb[k][:], GROUP_TOK, GROUP_TOK, DIM,
                )
                for c in range(CHUNKS_PER_GROUP):
                    tok0 = k * GROUP_TOK + c * P
                    lora_ps = ps.tile([P, DIM], dtype=f32, tag="lora_ps")
                    for h in range(2):
                        cs = slice(h * 512, (h + 1) * 512)
                        nc.tensor.matmul(
                            out=lora_ps[:, cs],
                            lhsT=a_t[:, tok0 : tok0 + P],
                            rhs=bt_sb[:, cs],
                            start=True, stop=True,
                        )
                    o = op.tile([P, DIM], dtype=f32, tag="o")
                    nc.vector.tensor_add(out=o[:], in0=g[:, c, :], in1=lora_ps[:])
                    nc.sync.dma_start(out=out[tok0 : tok0 + P, :], in_=o[:])

    nc.finalize()
    return nc


def kernel(input, emb_weight, lora_A, lora_B):
    import ml_dtypes
    from concourse.bass_utils import run_bass_kernel_spmd

    ids = np.asarray(input).astype(np.int64).reshape(-1)  # (16384,)
    emb_weight = np.asarray(emb_weight, dtype=np.float32)
    lora_A = np.asarray(lora_A, dtype=np.float32)
    lora_B = np.asarray(lora_B, dtype=np.float32)

    uniq, inv = np.unique(ids, return_inverse=True)
    u_rows = len(uniq)
    emb_small = np.ascontiguousarray(emb_weight[uniq])
    if EMB_BF16:
        emb_small = emb_small.astype(ml_dtypes.bfloat16)
    bt_host = np.ascontiguousarray((lora_B * SCALING).T).astype(ml_dtypes.bfloat16)

    inv16 = inv.astype(np.int16)
    in_maps = []
    for c in range(N_CORES):
        sl = inv16[c * TOK_PER_CORE : (c + 1) * TOK_PER_CORE]
        # wrapped idx layout per group: token j at [j % 16, j // 16],
        # replicated over the 8 GPSIMD core partition-groups
        idx_core = np.stack(
            [
                np.tile(sl[k * GROUP_TOK : (k + 1) * GROUP_TOK].reshape(-1, 16).T, (8, 1))
                for k in range(N_GROUPS)
            ]
        )
        at_core = np.ascontiguousarray(
            lora_A[:, ids[c * TOK_PER_CORE : (c + 1) * TOK_PER_CORE]]
        ).astype(ml_dtypes.bfloat16)
        in_maps.append({"idx": idx_core, "emb": emb_small, "at": at_core, "bt": bt_host})

    nc = _build_graph(u_rows)
    res = run_bass_kernel_spmd(nc, in_maps, list(range(N_CORES)))
    out = np.concatenate([res.results[i]["out"] for i in range(N_CORES)], axis=0)
    return out.reshape(B, S, DIM)


# revision 14
# speedup vs baseline: 1.8421x; 1.0728x over previous
"""Trainium2 Bass kernel for AdaptedEmbedding (embedding gather + LoRA).

out[b,s,:] = emb_weight[input[b,s], :] + (lora_A[:, input[b,s]].T @ lora_B.T) * (alpha/r)

Strategy (data-parallel over tokens, no collectives):
  Host:
    - Compact the vocab: only the unique tokens of this batch (U <= 16384)
      are shipped; all device indices then fit in int16 for dma_gather.
    - emb_small[U, 1024] f32; bt[16, 1024] bf16 = (lora_B * scaling)^T.
    - Per core: the (tiny, 64KB) per-token A^T slice is shipped dense and
      pre-transposed as at[16, 2048] bf16 -- the heavy gather (embedding
      rows, 8MB/core) runs on device.
    - Shard the 16384 tokens contiguously: 2048 tokens per core.
  Device (per core):
    - 8 dma_gathers of 256 tokens each pull embedding rows (pipelined,
      4 buffers deep).
    - Per 128-token chunk: 2 bf16 matmuls (K=16) with bt into PSUM, one
      1024-wide f32 vector-add (gathered + lora) into a staging tile,
      per-chunk DMA writeback.
  Host: concatenate per-core outputs -> (4, 4096, 1024).
"""

import numpy as np

B, S = 4, 4096
DIM = 1024
R = 16
SCALING = 2.0
N_CORES = 8
TOK_PER_CORE = (B * S) // N_CORES  # 2048
P = 128
N_GROUPS = 8
GROUP_TOK = TOK_PER_CORE // N_GROUPS  # 256
CHUNKS_PER_GROUP = GROUP_TOK // P  # 2
GROUP_IDX_COLS = GROUP_TOK // 16  # 16
EMB_BF16 = True  # bf16 embedding table: halves gather traffic, rel err ~1e-3


def _build_graph(u_rows: int):
    import concourse.bacc as bacc
    import concourse.mybir as mybir
    import concourse.tile as tile

    f32 = mybir.dt.float32
    bf16 = mybir.dt.bfloat16
    i16 = mybir.dt.int16

    nc = bacc.Bacc("TRN2", target_bir_lowering=False)

    emb_dt = bf16 if EMB_BF16 else f32
    idx = nc.declare_dram_parameter(
        "idx", [P, N_GROUPS * GROUP_IDX_COLS], i16, isOutput=False
    )
    emb = nc.declare_dram_parameter("emb", [u_rows, DIM], emb_dt, isOutput=False)
    at = nc.declare_dram_parameter("at", [R, TOK_PER_CORE], bf16, isOutput=False)
    bt = nc.declare_dram_parameter("bt", [R, DIM], bf16, isOutput=False)
    out = nc.declare_dram_parameter("out", [TOK_PER_CORE, DIM], f32, isOutput=True)

    with tile.TileContext(nc) as tc:
        with (
            tc.tile_pool(name="persist", bufs=1) as pers,
            tc.tile_pool(name="sbuf", bufs=6) as sb,
            tc.tile_pool(name="outp", bufs=4) as op,
            tc.tile_pool(name="psum", bufs=4, space="PSUM") as ps,
        ):
            idx_sb = pers.tile([P, N_GROUPS * GROUP_IDX_COLS], dtype=i16)
            nc.sync.dma_start(out=idx_sb[:], in_=idx[:])
            bt_sb = pers.tile([R, DIM], dtype=bf16)
            nc.scalar.dma_start(out=bt_sb[:], in_=bt[:])
            a_t = pers.tile([R, TOK_PER_CORE], dtype=bf16)
            nc.scalar.dma_start(out=a_t[:], in_=at[:])

            for k in range(N_GROUPS):
                g = sb.tile([P, CHUNKS_PER_GROUP, DIM], dtype=emb_dt, tag="g")
                nc.gpsimd.dma_gather(
                    g[:], emb[:],
                    idx_sb[:, k * GROUP_IDX_COLS : (k + 1) * GROUP_IDX_COLS],
                    GROUP_TOK, GROUP_TOK, DIM,
                )
                for c in range(CHUNKS_PER_GROUP):
                    tok0 = k * GROUP_TOK + c * P
                    lora_ps = ps.tile([P, DIM], dtype=f32, tag="lora_ps")
                    for h in range(2):
                        cs = slice(h * 512, (h + 1) * 512)
                        nc.tensor.matmul(
                            out=lora_ps[:, cs],
                            lhsT=a_t[:, tok0 : tok0 + P],
                            rhs=bt_sb[:, cs],
                            start=True, stop=True,
                        )
                    o = op.tile([P, DIM], dtype=f32, tag="o")
                    nc.vector.tensor_add(out=o[:], in0=g[:, c, :], in1=lora_ps[:])
                    nc.sync.dma_start(out=out[tok0 : tok0 + P, :], in_=o[:])

    nc.finalize()
    return nc


def kernel(input, emb_weight, lora_A, lora_B):
    import ml_dtypes
    from concourse.bass_utils import run_bass_kernel_spmd

    ids = np.asarray(input).astype(np.int64).reshape(-1)  # (16384,)
    emb_weight = np.asarray(emb_weight, dtype=np.float32)
    lora_A = np.asarray(lora_A, dtype=np.float32)
    lora_B = np.asarray(lora_B, dtype=np.float32)

    uniq, inv = np.unique(ids, return_inverse=True)
    u_rows = len(uniq)
    emb_small = np.ascontiguousarray(emb_weight[uniq])
    if EMB_BF16:
        emb_small = emb_small.astype(ml_dtypes.bfloat16)
    bt_host = np.ascontiguousarray((lora_B * SCALING).T).astype(ml_dtypes.bfloat16)

    inv16 = inv.astype(np.int16)
    in_maps = []
    for c in range(N_CORES):
        sl = inv16[c * TOK_PER_CORE : (c + 1) * TOK_PER_CORE]
        # wrapped idx layout per group: token j at [j % 16, j // 16],
        # replicated over the 8 GPSIMD core partition-groups; groups are
        # side-by-side column blocks of one tile
        idx_core = np.concatenate(
            [
                np.tile(sl[k * GROUP_TOK : (k + 1) * GROUP_TOK].reshape(-1, 16).T, (8, 1))
                for k in range(N_GROUPS)
            ],
            axis=1,
        )
        at_core = np.ascontiguousarray(
            lora_A[:, ids[c * TOK_PER_CORE : (c + 1) * TOK_PER_CORE]]
        ).astype(ml_dtypes.bfloat16)
        in_maps.append({"idx": idx_core, "emb": emb_small, "at": at_core, "bt": bt_host})

    nc = _build_graph(u_rows)
    res = run_bass_kernel_spmd(nc, in_maps, list(range(N_CORES)))
    out = np.concatenate([res.results[i]["out"] for i in range(N_CORES)], axis=0)
    return out.reshape(B, S, DIM)


# revision 18
# speedup vs baseline: 1.9149x; 1.0395x over previous
"""Trainium2 Bass kernel for AdaptedEmbedding (embedding gather + LoRA).

out[b,s,:] = emb_weight[input[b,s], :] + (lora_A[:, input[b,s]].T @ lora_B.T) * (alpha/r)

Strategy (data-parallel over tokens, no collectives):
  Host:
    - Compact the vocab: only the unique tokens of this batch (U <= 16384)
      are shipped; all device indices then fit in int16 for dma_gather.
    - emb_small[U, 1024] f32; bt[16, 1024] bf16 = (lora_B * scaling)^T.
    - Per core: the (tiny, 64KB) per-token A^T slice is shipped dense and
      pre-transposed as at[16, 2048] bf16 -- the heavy gather (embedding
      rows, 8MB/core) runs on device.
    - Shard the 16384 tokens contiguously: 2048 tokens per core.
  Device (per core):
    - 8 dma_gathers of 256 tokens each pull embedding rows (pipelined,
      4 buffers deep).
    - Per 128-token chunk: 2 bf16 matmuls (K=16) with bt into PSUM, one
      1024-wide f32 vector-add (gathered + lora) into a staging tile,
      per-chunk DMA writeback.
  Host: concatenate per-core outputs -> (4, 4096, 1024).
"""

import numpy as np

B, S = 4, 4096
DIM = 1024
R = 16
SCALING = 2.0
N_CORES = 8
TOK_PER_CORE = (B * S) // N_CORES  # 2048
P = 128
N_GROUPS = 8
GROUP_TOK = TOK_PER_CORE // N_GROUPS  # 256
CHUNKS_PER_GROUP = GROUP_TOK // P  # 2
GROUP_IDX_COLS = GROUP_TOK // 16  # 16
EMB_BF16 = True  # bf16 embedding table: halves gather traffic, rel err ~1e-3


def _build_graph(u_rows: int):
    import concourse.bacc as bacc
    import concourse.mybir as mybir
    import concourse.tile as tile

    f32 = mybir.dt.float32
    bf16 = mybir.dt.bfloat16
    i16 = mybir.dt.int16

    nc = bacc.Bacc("TRN2", target_bir_lowering=False)

    emb_dt = bf16 if EMB_BF16 else f32
    idx = nc.declare_dram_parameter(
        "idx", [P, N_GROUPS * GROUP_IDX_COLS], i16, isOutput=False
    )
    emb = nc.declare_dram_parameter("emb", [u_rows, DIM], emb_dt, isOutput=False)
    at = nc.declare_dram_parameter("at", [R, TOK_PER_CORE], bf16, isOutput=False)
    bt = nc.declare_dram_parameter("bt", [R, DIM], bf16, isOutput=False)
    out = nc.declare_dram_parameter("out", [TOK_PER_CORE, DIM], bf16, isOutput=True)

    with tile.TileContext(nc) as tc:
        with (
            tc.tile_pool(name="persist", bufs=1) as pers,
            tc.tile_pool(name="sbuf", bufs=6) as sb,
            tc.tile_pool(name="outp", bufs=4) as op,
            tc.tile_pool(name="psum", bufs=4, space="PSUM") as ps,
        ):
            # warmup gather: absorbs the GPSIMD custom-ucode load / pool
            # reconfig latency while the real idx tile is still in flight
            warm_idx = pers.tile([P, 1], dtype=i16)
            nc.gpsimd.memset(warm_idx[:], 0)
            warm_out = pers.tile([P, 1, DIM], dtype=emb_dt)
            nc.gpsimd.dma_gather(warm_out[:], emb[:], warm_idx[:], 16, 16, DIM)

            idx_sb = pers.tile([P, N_GROUPS * GROUP_IDX_COLS], dtype=i16)
            nc.sync.dma_start(out=idx_sb[:], in_=idx[:])
            bt_sb = pers.tile([R, DIM], dtype=bf16)
            nc.scalar.dma_start(out=bt_sb[:], in_=bt[:])
            a_t = pers.tile([R, TOK_PER_CORE], dtype=bf16)
            nc.scalar.dma_start(out=a_t[:], in_=at[:])

            for k in range(N_GROUPS):
                g = sb.tile([P, CHUNKS_PER_GROUP, DIM], dtype=emb_dt, tag="g")
                nc.gpsimd.dma_gather(
                    g[:], emb[:],
                    idx_sb[:, k * GROUP_IDX_COLS : (k + 1) * GROUP_IDX_COLS],
                    GROUP_TOK, GROUP_TOK, DIM,
                )
                for c in range(CHUNKS_PER_GROUP):
                    tok0 = k * GROUP_TOK + c * P
                    lora_ps = ps.tile([P, DIM], dtype=f32, tag="lora_ps")
                    for h in range(2):
                        cs = slice(h * 512, (h + 1) * 512)
                        nc.tensor.matmul(
                            out=lora_ps[:, cs],
                            lhsT=a_t[:, tok0 : tok0 + P],
                            rhs=bt_sb[:, cs],
                            start=True, stop=True,
                        )
                    o = op.tile([P, DIM], dtype=bf16, tag="o")
                    nc.vector.tensor_add(out=o[:], in0=g[:, c, :], in1=lora_ps[:])
                    nc.sync.dma_start(out=out[tok0 : tok0 + P, :], in_=o[:])

    nc.finalize()
    return nc


def kernel(input, emb_weight, lora_A, lora_B):
    import ml_dtypes
    from concourse.bass_utils import run_bass_kernel_spmd

    ids = np.asarray(input).astype(np.int64).reshape(-1)  # (16384,)
    emb_weight = np.asarray(emb_weight, dtype=np.float32)
    lora_A = np.asarray(lora_A, dtype=np.float32)
    lora_B = np.asarray(lora_B, dtype=np.float32)

    uniq, inv = np.unique(ids, return_inverse=True)
    u_rows = len(uniq)
    emb_small = np.ascontiguousarray(emb_weight[uniq])
    if EMB_BF16:
        emb_small = emb_small.astype(ml_dtypes.bfloat16)
    bt_host = np.ascontiguousarray((lora_B * SCALING).T).astype(ml_dtypes.bfloat16)

    inv16 = inv.astype(np.int16)
    in_maps = []
    for c in range(N_CORES):
        sl = inv16[c * TOK_PER_CORE : (c + 1) * TOK_PER_CORE]
        # wrapped idx layout per group: token j at [j % 16, j // 16],
        # replicated over the 8 GPSIMD core partition-groups; groups are
        # side-by-side column blocks of one tile
        idx_core = np.concatenate(
            [
                np.tile(sl[k * GROUP_TOK : (k + 1) * GROUP_TOK].reshape(-1, 16).T, (8, 1))
                for k in range(N_GROUPS)
            ],
            axis=1,
        )
        at_core = np.ascontiguousarray(
            lora_A[:, ids[c * TOK_PER_CORE : (c + 1) * TOK_PER_CORE]]
        ).astype(ml_dtypes.bfloat16)
        in_maps.append({"idx": idx_core, "emb": emb_small, "at": at_core, "bt": bt_host})

    nc = _build_graph(u_rows)
    res = run_bass_kernel_spmd(nc, in_maps, list(range(N_CORES)))
    out = np.concatenate(
        [np.asarray(res.results[i]["out"]).astype(np.float32) for i in range(N_CORES)],
        axis=0,
    )
    return out.reshape(B, S, DIM)


# revision 20
# speedup vs baseline: 1.9793x; 1.0336x over previous
"""Trainium2 Bass kernel for AdaptedEmbedding (embedding gather + LoRA).

out[b,s,:] = emb_weight[input[b,s], :] + (lora_A[:, input[b,s]].T @ lora_B.T) * (alpha/r)

Strategy (data-parallel over tokens, no collectives):
  Host:
    - Compact the vocab: only the unique tokens of this batch (U <= 16384)
      are shipped; all device indices then fit in int16 for dma_gather.
    - emb_small[U, 1024] f32; bt[16, 1024] bf16 = (lora_B * scaling)^T.
    - Per core: the (tiny, 64KB) per-token A^T slice is shipped dense and
      pre-transposed as at[16, 2048] bf16 -- the heavy gather (embedding
      rows, 8MB/core) runs on device.
    - Shard the 16384 tokens contiguously: 2048 tokens per core.
  Device (per core):
    - 8 dma_gathers of 256 tokens each pull embedding rows (pipelined,
      4 buffers deep).
    - Per 128-token chunk: 2 bf16 matmuls (K=16) with bt into PSUM, one
      1024-wide f32 vector-add (gathered + lora) into a staging tile,
      per-chunk DMA writeback.
  Host: concatenate per-core outputs -> (4, 4096, 1024).
"""

import numpy as np

B, S = 4, 4096
DIM = 1024
R = 16
SCALING = 2.0
N_CORES = 8
TOK_PER_CORE = (B * S) // N_CORES  # 2048
P = 128
N_GROUPS = 8
GROUP_TOK = TOK_PER_CORE // N_GROUPS  # 256
CHUNKS_PER_GROUP = GROUP_TOK // P  # 2
GROUP_IDX_COLS = GROUP_TOK // 16  # 16
EMB_BF16 = True  # bf16 embedding table: halves gather traffic, rel err ~1e-3


def _build_graph(u_rows: int):
    import concourse.bacc as bacc
    import concourse.mybir as mybir
    import concourse.tile as tile

    f32 = mybir.dt.float32
    bf16 = mybir.dt.bfloat16
    i16 = mybir.dt.int16

    nc = bacc.Bacc("TRN2", target_bir_lowering=False)

    emb_dt = bf16 if EMB_BF16 else f32
    idx = nc.declare_dram_parameter(
        "idx", [P, N_GROUPS * GROUP_IDX_COLS], i16, isOutput=False
    )
    emb = nc.declare_dram_parameter("emb", [u_rows, DIM], emb_dt, isOutput=False)
    at = nc.declare_dram_parameter("at", [R, TOK_PER_CORE], bf16, isOutput=False)
    bt = nc.declare_dram_parameter("bt", [R, DIM], bf16, isOutput=False)
    out = nc.declare_dram_parameter("out", [TOK_PER_CORE, DIM], bf16, isOutput=True)

    with tile.TileContext(nc) as tc:
        with (
            tc.tile_pool(name="persist", bufs=1) as pers,
            tc.tile_pool(name="sbuf", bufs=6) as sb,
            tc.tile_pool(name="outp", bufs=4) as op,
            tc.tile_pool(name="psum", bufs=4, space="PSUM") as ps,
        ):
            idx_sb = pers.tile([P, N_GROUPS * GROUP_IDX_COLS], dtype=i16)
            nc.sync.dma_start(out=idx_sb[:], in_=idx[:])
            bt_sb = pers.tile([R, DIM], dtype=bf16)
            nc.scalar.dma_start(out=bt_sb[:], in_=bt[:])
            a_t = pers.tile([R, TOK_PER_CORE], dtype=bf16)
            nc.scalar.dma_start(out=a_t[:], in_=at[:])

            ntok_reg = nc.gpsimd.to_reg(GROUP_TOK)
            for k in range(N_GROUPS):
                g = sb.tile([P, CHUNKS_PER_GROUP, DIM], dtype=emb_dt, tag="g")
                nc.gpsimd.dma_gather(
                    g[:], emb[:],
                    idx_sb[:, k * GROUP_IDX_COLS : (k + 1) * GROUP_IDX_COLS],
                    GROUP_TOK, ntok_reg, DIM,
                )
                for c in range(CHUNKS_PER_GROUP):
                    tok0 = k * GROUP_TOK + c * P
                    lora_ps = ps.tile([P, DIM], dtype=f32, tag="lora_ps")
                    for h in range(2):
                        cs = slice(h * 512, (h + 1) * 512)
                        nc.tensor.matmul(
                            out=lora_ps[:, cs],
                            lhsT=a_t[:, tok0 : tok0 + P],
                            rhs=bt_sb[:, cs],
                            start=True, stop=True,
                        )
                    o = op.tile([P, DIM], dtype=bf16, tag="o")
                    nc.vector.tensor_add(out=o[:], in0=g[:, c, :], in1=lora_ps[:])
                    nc.sync.dma_start(out=out[tok0 : tok0 + P, :], in_=o[:])

    nc.finalize()
    return nc


def kernel(input, emb_weight, lora_A, lora_B):
    import ml_dtypes
    from concourse.bass_utils import run_bass_kernel_spmd

    ids = np.asarray(input).astype(np.int64).reshape(-1)  # (16384,)
    emb_weight = np.asarray(emb_weight, dtype=np.float32)
    lora_A = np.asarray(lora_A, dtype=np.float32)
    lora_B = np.asarray(lora_B, dtype=np.float32)

    uniq, inv = np.unique(ids, return_inverse=True)
    u_rows = len(uniq)
    emb_small = np.ascontiguousarray(emb_weight[uniq])
    if EMB_BF16:
        emb_small = emb_small.astype(ml_dtypes.bfloat16)
    bt_host = np.ascontiguousarray((lora_B * SCALING).T).astype(ml_dtypes.bfloat16)

    inv16 = inv.astype(np.int16)
    in_maps = []
    for c in range(N_CORES):
        sl = inv16[c * TOK_PER_CORE : (c + 1) * TOK_PER_CORE]
        # wrapped idx layout per group: token j at [j % 16, j // 16],
        # replicated over the 8 GPSIMD core partition-groups; groups are
        # side-by-side column blocks of one tile
        idx_core = np.concatenate(
            [
                np.tile(sl[k * GROUP_TOK : (k + 1) * GROUP_TOK].reshape(-1, 16).T, (8, 1))
                for k in range(N_GROUPS)
            ],
            axis=1,
        )
        at_core = np.ascontiguousarray(
            lora_A[:, ids[c * TOK_PER_CORE : (c + 1) * TOK_PER_CORE]]
        ).astype(ml_dtypes.bfloat16)
        in_maps.append({"idx": idx_core, "emb": emb_small, "at": at_core, "bt": bt_host})

    nc = _build_graph(u_rows)
    res = run_bass_kernel_spmd(nc, in_maps, list(range(N_CORES)))
    out = np.concatenate(
        [np.asarray(res.results[i]["out"]).astype(np.float32) for i in range(N_CORES)],
        axis=0,
    )
    return out.reshape(B, S, DIM)


# revision 24
# speedup vs baseline: 1.9840x; 1.0024x over previous
"""Trainium2 Bass kernel for AdaptedEmbedding (embedding gather + LoRA).

out[b,s,:] = emb_weight[input[b,s], :] + (lora_A[:, input[b,s]].T @ lora_B.T) * (alpha/r)

Strategy (data-parallel over tokens, no collectives):
  Host:
    - Compact the vocab: only the unique tokens of this batch (U <= 16384)
      are shipped; all device indices then fit in int16 for dma_gather.
    - emb_small[U, 1024] f32; bt[16, 1024] bf16 = (lora_B * scaling)^T.
    - Per core: the (tiny, 64KB) per-token A^T slice is shipped dense and
      pre-transposed as at[16, 2048] bf16 -- the heavy gather (embedding
      rows, 8MB/core) runs on device.
    - Shard the 16384 tokens contiguously: 2048 tokens per core.
  Device (per core):
    - 8 dma_gathers of 256 tokens each pull embedding rows (pipelined,
      4 buffers deep).
    - Per 128-token chunk: 2 bf16 matmuls (K=16) with bt into PSUM, one
      1024-wide f32 vector-add (gathered + lora) into a staging tile,
      per-chunk DMA writeback.
  Host: concatenate per-core outputs -> (4, 4096, 1024).
"""

import numpy as np

B, S = 4, 4096
DIM = 1024
R = 16
SCALING = 2.0
N_CORES = 8
TOK_PER_CORE = (B * S) // N_CORES  # 2048
P = 128
N_GROUPS = 8
GROUP_TOK = TOK_PER_CORE // N_GROUPS  # 256
CHUNKS_PER_GROUP = GROUP_TOK // P  # 2
GROUP_IDX_COLS = GROUP_TOK // 16  # 16
EMB_BF16 = True  # bf16 embedding table: halves gather traffic, rel err ~1e-3


def _build_graph(u_rows: int):
    import concourse.bacc as bacc
    import concourse.bass as bass
    import concourse.mybir as mybir
    import concourse.tile as tile

    f32 = mybir.dt.float32
    bf16 = mybir.dt.bfloat16
    i16 = mybir.dt.int16

    nc = bacc.Bacc("TRN2", target_bir_lowering=False)

    emb_dt = bf16 if EMB_BF16 else f32
    idx = nc.declare_dram_parameter(
        "idx", [P, N_GROUPS * GROUP_IDX_COLS], i16, isOutput=False
    )
    idx32 = nc.declare_dram_parameter(
        "idx32", [P, TOK_PER_CORE // P], mybir.dt.int32, isOutput=False
    )
    emb = nc.declare_dram_parameter("emb", [u_rows, DIM], emb_dt, isOutput=False)
    at = nc.declare_dram_parameter("at", [R, TOK_PER_CORE], bf16, isOutput=False)
    bt = nc.declare_dram_parameter("bt", [R, DIM], bf16, isOutput=False)
    out = nc.declare_dram_parameter("out", [TOK_PER_CORE, DIM], bf16, isOutput=True)

    with tile.TileContext(nc) as tc:
        with (
            tc.tile_pool(name="persist", bufs=1) as pers,
            tc.tile_pool(name="sbuf", bufs=6) as sb,
            tc.tile_pool(name="outp", bufs=4) as op,
            tc.tile_pool(name="psum", bufs=4, space="PSUM") as ps,
        ):
            idx_sb = pers.tile([P, N_GROUPS * GROUP_IDX_COLS], dtype=i16)
            nc.sync.dma_start(out=idx_sb[:], in_=idx[:])
            idx32_sb = pers.tile([P, TOK_PER_CORE // P], dtype=mybir.dt.int32)
            nc.sync.dma_start(out=idx32_sb[:], in_=idx32[:])
            bt_sb = pers.tile([R, DIM], dtype=bf16)
            nc.scalar.dma_start(out=bt_sb[:], in_=bt[:])
            a_t = pers.tile([R, TOK_PER_CORE], dtype=bf16)
            nc.scalar.dma_start(out=a_t[:], in_=at[:])

            ntok_reg = nc.gpsimd.to_reg(GROUP_TOK)

            def compute_chunk(g_chunk_ap, tok0):
                lora_ps = ps.tile([P, DIM], dtype=f32, tag="lora_ps")
                for h in range(2):
                    cs = slice(h * 512, (h + 1) * 512)
                    nc.tensor.matmul(
                        out=lora_ps[:, cs],
                        lhsT=a_t[:, tok0 : tok0 + P],
                        rhs=bt_sb[:, cs],
                        start=True, stop=True,
                    )
                o = op.tile([P, DIM], dtype=bf16, tag="o")
                nc.vector.tensor_add(out=o[:], in0=g_chunk_ap, in1=lora_ps[:])
                nc.sync.dma_start(out=out[tok0 : tok0 + P, :], in_=o[:])

            # first half: dma_gather (extended-DGE queue)
            for k in range(N_GROUPS // 2):
                g = sb.tile([P, CHUNKS_PER_GROUP, DIM], dtype=emb_dt, tag="g")
                nc.gpsimd.dma_gather(
                    g[:], emb[:],
                    idx_sb[:, k * GROUP_IDX_COLS : (k + 1) * GROUP_IDX_COLS],
                    GROUP_TOK, ntok_reg, DIM,
                )
                for c in range(CHUNKS_PER_GROUP):
                    compute_chunk(g[:, c, :], k * GROUP_TOK + c * P)

            # second half: per-chunk indirect gathers (qPoolDynamic)
            for ch in range(TOK_PER_CORE // (2 * P), TOK_PER_CORE // P):
                g2 = sb.tile([P, DIM], dtype=emb_dt, tag="g2")
                nc.gpsimd.indirect_dma_start(
                    out=g2[:],
                    out_offset=None,
                    in_=emb[:],
                    in_offset=bass.IndirectOffsetOnAxis(
                        ap=idx32_sb[:, ch : ch + 1], axis=0
                    ),
                )
                compute_chunk(g2[:], ch * P)

    nc.finalize()
    return nc


def kernel(input, emb_weight, lora_A, lora_B):
    import ml_dtypes
    from concourse.bass_utils import run_bass_kernel_spmd

    ids = np.asarray(input).astype(np.int64).reshape(-1)  # (16384,)
    emb_weight = np.asarray(emb_weight, dtype=np.float32)
    lora_A = np.asarray(lora_A, dtype=np.float32)
    lora_B = np.asarray(lora_B, dtype=np.float32)

    uniq, inv = np.unique(ids, return_inverse=True)
    u_rows = len(uniq)
    emb_small = np.ascontiguousarray(emb_weight[uniq])
    if EMB_BF16:
        emb_small = emb_small.astype(ml_dtypes.bfloat16)
    bt_host = np.ascontiguousarray((lora_B * SCALING).T).astype(ml_dtypes.bfloat16)

    inv16 = inv.astype(np.int16)
    in_maps = []
    for c in range(N_CORES):
        sl = inv16[c * TOK_PER_CORE : (c + 1) * TOK_PER_CORE]
        # wrapped idx layout per group: token j at [j % 16, j // 16],
        # replicated over the 8 GPSIMD core partition-groups; groups are
        # side-by-side column blocks of one tile
        idx_core = np.concatenate(
            [
                np.tile(sl[k * GROUP_TOK : (k + 1) * GROUP_TOK].reshape(-1, 16).T, (8, 1))
                for k in range(N_GROUPS)
            ],
            axis=1,
        )
        at_core = np.ascontiguousarray(
            lora_A[:, ids[c * TOK_PER_CORE : (c + 1) * TOK_PER_CORE]]
        ).astype(ml_dtypes.bfloat16)
        # int32 idx for the indirect-gather chunks: [partition, chunk]
        idx32_core = np.ascontiguousarray(
            sl.astype(np.int32).reshape(TOK_PER_CORE // P, P).T
        )
        in_maps.append(
            {"idx": idx_core, "idx32": idx32_core, "emb": emb_small,
             "at": at_core, "bt": bt_host}
        )

    nc = _build_graph(u_rows)
    res = run_bass_kernel_spmd(nc, in_maps, list(range(N_CORES)))
    out = np.concatenate(
        [np.asarray(res.results[i]["out"]).astype(np.float32) for i in range(N_CORES)],
        axis=0,
    )
    return out.reshape(B, S, DIM)


# revision 26
# speedup vs baseline: 2.3597x; 1.1894x over previous
"""Trainium2 Bass kernel for AdaptedEmbedding (embedding gather + LoRA).

out[b,s,:] = emb_weight[input[b,s], :] + (lora_A[:, input[b,s]].T @ lora_B.T) * (alpha/r)

Strategy (data-parallel over tokens, no collectives):
  Host:
    - Compact the vocab: only the unique tokens of this batch (U <= 16384)
      rows are shipped, in bf16 (rel err ~2e-3, tolerance 2e-2).
    - bt[16, 1024] bf16 = (lora_B * scaling)^T.
    - Per core: the (tiny, 64KB) per-token A^T slice is shipped dense and
      pre-transposed as at[16, 2048] bf16 -- the heavy gather (embedding
      rows, 4MB/core) runs on device.
    - Shard the 16384 tokens contiguously: 2048 tokens per core.
  Device (per core):
    - 16 indirect DMA gathers (128 rows each) pull embedding rows,
      deep-buffered; descriptor generation on the Q7 SWDGE is the pacer.
    - Per 128-token chunk: 2 bf16 matmuls (K=16) with bt into PSUM.
    - Per 256-token group: one 2048-wide vector add (gathered + lora)
      into a bf16 staging tile, one DMA writeback.
  Host: concatenate per-core outputs, upcast to f32 -> (4, 4096, 1024).
"""

import numpy as np

B, S = 4, 4096
DIM = 1024
R = 16
SCALING = 2.0
N_CORES = 8
TOK_PER_CORE = (B * S) // N_CORES  # 2048
P = 128
N_GROUPS = 8
GROUP_TOK = TOK_PER_CORE // N_GROUPS  # 256
CHUNKS_PER_GROUP = GROUP_TOK // P  # 2
N_CHUNKS = TOK_PER_CORE // P  # 16
EMB_BF16 = True  # bf16 embedding table: halves gather traffic


def _build_graph(u_rows: int):
    import concourse.bacc as bacc
    import concourse.bass as bass
    import concourse.mybir as mybir
    import concourse.tile as tile

    f32 = mybir.dt.float32
    bf16 = mybir.dt.bfloat16
    emb_dt = bf16 if EMB_BF16 else f32

    nc = bacc.Bacc("TRN2", target_bir_lowering=False)

    idx32 = nc.declare_dram_parameter("idx32", [P, N_CHUNKS], mybir.dt.int32, isOutput=False)
    emb = nc.declare_dram_parameter("emb", [u_rows, DIM], emb_dt, isOutput=False)
    at = nc.declare_dram_parameter("at", [R, TOK_PER_CORE], bf16, isOutput=False)
    bt = nc.declare_dram_parameter("bt", [R, DIM], bf16, isOutput=False)
    out = nc.declare_dram_parameter("out", [TOK_PER_CORE, DIM], bf16, isOutput=True)

    with tile.TileContext(nc) as tc:
        with (
            tc.tile_pool(name="persist", bufs=1) as pers,
            tc.tile_pool(name="sbuf", bufs=6) as sb,
            tc.tile_pool(name="outp", bufs=3) as op,
            tc.tile_pool(name="psum", bufs=2, space="PSUM") as ps,
        ):
            idx_sb = pers.tile([P, N_CHUNKS], dtype=mybir.dt.int32)
            nc.sync.dma_start(out=idx_sb[:], in_=idx32[:])
            bt_sb = pers.tile([R, DIM], dtype=bf16)
            nc.scalar.dma_start(out=bt_sb[:], in_=bt[:])
            a_t = pers.tile([R, TOK_PER_CORE], dtype=bf16)
            nc.scalar.dma_start(out=a_t[:], in_=at[:])

            for k in range(N_GROUPS):
                g = sb.tile([P, CHUNKS_PER_GROUP * DIM], dtype=emb_dt, tag="g")
                lora_ps = ps.tile([P, CHUNKS_PER_GROUP * DIM], dtype=f32, tag="lora_ps")
                for c in range(CHUNKS_PER_GROUP):
                    ch = k * CHUNKS_PER_GROUP + c
                    nc.gpsimd.indirect_dma_start(
                        out=g[:, c * DIM : (c + 1) * DIM],
                        out_offset=None,
                        in_=emb[:],
                        in_offset=bass.IndirectOffsetOnAxis(
                            ap=idx_sb[:, ch : ch + 1], axis=0
                        ),
                    )
                    tok0 = ch * P
                    for h in range(2):
                        nc.tensor.matmul(
                            out=lora_ps[:, c * DIM + h * 512 : c * DIM + (h + 1) * 512],
                            lhsT=a_t[:, tok0 : tok0 + P],
                            rhs=bt_sb[:, h * 512 : (h + 1) * 512],
                            start=True, stop=True,
                        )
                o = op.tile([P, CHUNKS_PER_GROUP * DIM], dtype=bf16, tag="o")
                nc.vector.tensor_add(out=o[:], in0=g[:], in1=lora_ps[:])
                nc.sync.dma_start(
                    out=out[k * GROUP_TOK : (k + 1) * GROUP_TOK, :].rearrange(
                        "(c p) d -> p c d", p=P
                    ),
                    in_=o[:].rearrange("p (c d) -> p c d", d=DIM),
                )

    nc.finalize()
    return nc


def kernel(input, emb_weight, lora_A, lora_B):
    import ml_dtypes
    from concourse.bass_utils import run_bass_kernel_spmd

    ids = np.asarray(input).astype(np.int64).reshape(-1)  # (16384,)
    emb_weight = np.asarray(emb_weight, dtype=np.float32)
    lora_A = np.asarray(lora_A, dtype=np.float32)
    lora_B = np.asarray(lora_B, dtype=np.float32)

    uniq, inv = np.unique(ids, return_inverse=True)
    u_rows = len(uniq)
    emb_small = np.ascontiguousarray(emb_weight[uniq])
    if EMB_BF16:
        emb_small = emb_small.astype(ml_dtypes.bfloat16)
    bt_host = np.ascontiguousarray((lora_B * SCALING).T).astype(ml_dtypes.bfloat16)

    in_maps = []
    for c in range(N_CORES):
        sl = inv[c * TOK_PER_CORE : (c + 1) * TOK_PER_CORE]
        # int32 idx layout for indirect gathers: [partition, chunk]
        idx32_core = np.ascontiguousarray(sl.astype(np.int32).reshape(N_CHUNKS, P).T)
        at_core = np.ascontiguousarray(
            lora_A[:, ids[c * TOK_PER_CORE : (c + 1) * TOK_PER_CORE]]
        ).astype(ml_dtypes.bfloat16)
        in_maps.append(
            {"idx32": idx32_core, "emb": emb_small, "at": at_core, "bt": bt_host}
        )

    nc = _build_graph(u_rows)
    res = run_bass_kernel_spmd(nc, in_maps, list(range(N_CORES)))
    out = np.concatenate(
        [np.asarray(res.results[i]["out"]).astype(np.float32) for i in range(N_CORES)],
        axis=0,
    )
    return out.reshape(B, S, DIM)


# revision 27
# speedup vs baseline: 2.5416x; 1.0771x over previous
"""Trainium2 Bass kernel for AdaptedEmbedding (embedding gather + LoRA).

out[b,s,:] = emb_weight[input[b,s], :] + (lora_A[:, input[b,s]].T @ lora_B.T) * (alpha/r)

Strategy (data-parallel over tokens, no collectives):
  Host:
    - Compact the vocab: only the unique tokens of this batch (U <= 16384)
      rows are shipped, in bf16 (rel err ~2e-3, tolerance 2e-2).
    - bt[16, 1024] bf16 = (lora_B * scaling)^T.
    - Per core: the (tiny, 64KB) per-token A^T slice is shipped dense and
      pre-transposed as at[16, 2048] bf16 -- the heavy gather (embedding
      rows, 4MB/core) runs on device.
    - Shard the 16384 tokens contiguously: 2048 tokens per core.
  Device (per core):
    - 16 indirect DMA gathers (128 rows each) pull embedding rows,
      deep-buffered; descriptor generation on the Q7 SWDGE is the pacer.
    - Per 128-token chunk: 2 bf16 matmuls (K=16) with bt into PSUM.
    - Per 256-token group: one 2048-wide vector add (gathered + lora)
      into a bf16 staging tile, one DMA writeback.
  Host: concatenate per-core outputs, upcast to f32 -> (4, 4096, 1024).
"""

import numpy as np

B, S = 4, 4096
DIM = 1024
R = 16
SCALING = 2.0
N_CORES = 8
TOK_PER_CORE = (B * S) // N_CORES  # 2048
P = 128
N_GROUPS = 8
GROUP_TOK = TOK_PER_CORE // N_GROUPS  # 256
CHUNKS_PER_GROUP = GROUP_TOK // P  # 2
N_CHUNKS = TOK_PER_CORE // P  # 16
EMB_BF16 = True  # bf16 embedding table: halves gather traffic


def _build_graph(u_rows: int):
    import concourse.bacc as bacc
    import concourse.bass as bass
    import concourse.mybir as mybir
    import concourse.tile as tile

    f32 = mybir.dt.float32
    bf16 = mybir.dt.bfloat16
    emb_dt = bf16 if EMB_BF16 else f32

    nc = bacc.Bacc("TRN2", target_bir_lowering=False)

    idx32 = nc.declare_dram_parameter("idx32", [P, N_CHUNKS], mybir.dt.int32, isOutput=False)
    emb = nc.declare_dram_parameter("emb", [u_rows, DIM], emb_dt, isOutput=False)
    at = nc.declare_dram_parameter("at", [R, TOK_PER_CORE], bf16, isOutput=False)
    bt = nc.declare_dram_parameter("bt", [R, DIM], bf16, isOutput=False)
    out = nc.declare_dram_parameter("out", [TOK_PER_CORE, DIM], bf16, isOutput=True)

    with tile.TileContext(nc) as tc:
        with (
            tc.tile_pool(name="persist", bufs=1) as pers,
            tc.tile_pool(name="sbuf", bufs=8) as sb,
            tc.tile_pool(name="outp", bufs=4) as op,
            tc.tile_pool(name="psum", bufs=2, space="PSUM") as ps,
        ):
            idx_sb = pers.tile([P, N_CHUNKS], dtype=mybir.dt.int32)
            nc.gpsimd.dma_start(out=idx_sb[:], in_=idx32[:])
            bt_sb = pers.tile([R, DIM], dtype=bf16)
            nc.scalar.dma_start(out=bt_sb[:], in_=bt[:])
            a_t = pers.tile([R, TOK_PER_CORE], dtype=bf16)
            nc.scalar.dma_start(out=a_t[:], in_=at[:])

            for k in range(N_GROUPS):
                g = sb.tile([P, CHUNKS_PER_GROUP * DIM], dtype=emb_dt, tag="g")
                lora_ps = ps.tile([P, CHUNKS_PER_GROUP * DIM], dtype=f32, tag="lora_ps")
                for c in range(CHUNKS_PER_GROUP):
                    ch = k * CHUNKS_PER_GROUP + c
                    nc.gpsimd.indirect_dma_start(
                        out=g[:, c * DIM : (c + 1) * DIM],
                        out_offset=None,
                        in_=emb[:],
                        in_offset=bass.IndirectOffsetOnAxis(
                            ap=idx_sb[:, ch : ch + 1], axis=0
                        ),
                    )
                    tok0 = ch * P
                    for h in range(2):
                        nc.tensor.matmul(
                            out=lora_ps[:, c * DIM + h * 512 : c * DIM + (h + 1) * 512],
                            lhsT=a_t[:, tok0 : tok0 + P],
                            rhs=bt_sb[:, h * 512 : (h + 1) * 512],
                            start=True, stop=True,
                        )
                o = op.tile([P, CHUNKS_PER_GROUP * DIM], dtype=bf16, tag="o")
                nc.vector.tensor_add(out=o[:], in0=g[:], in1=lora_ps[:])
                nc.sync.dma_start(
                    out=out[k * GROUP_TOK : (k + 1) * GROUP_TOK, :].rearrange(
                        "(c p) d -> p c d", p=P
                    ),
                    in_=o[:].rearrange("p (c d) -> p c d", d=DIM),
                )

    nc.finalize()
    return nc


def kernel(input, emb_weight, lora_A, lora_B):
    import ml_dtypes
    from concourse.bass_utils import run_bass_kernel_spmd

    ids = np.asarray(input).astype(np.int64).reshape(-1)  # (16384,)
    emb_weight = np.asarray(emb_weight, dtype=np.float32)
    lora_A = np.asarray(lora_A, dtype=np.float32)
    lora_B = np.asarray(lora_B, dtype=np.float32)

    uniq, inv = np.unique(ids, return_inverse=True)
    u_rows = len(uniq)
    emb_small = np.ascontiguousarray(emb_weight[uniq])
    if EMB_BF16:
        emb_small = emb_small.astype(ml_dtypes.bfloat16)
    bt_host = np.ascontiguousarray((lora_B * SCALING).T).astype(ml_dtypes.bfloat16)

    in_maps = []
    for c in range(N_CORES):
        sl = inv[c * TOK_PER_CORE : (c + 1) * TOK_PER_CORE]
        # int32 idx layout for indirect gathers: [partition, chunk]
        idx32_core = np.ascontiguousarray(sl.astype(np.int32).reshape(N_CHUNKS, P).T)
        at_core = np.ascontiguousarray(
            lora_A[:, ids[c * TOK_PER_CORE : (c + 1) * TOK_PER_CORE]]
        ).astype(ml_dtypes.bfloat16)
        in_maps.append(
            {"idx32": idx32_core, "emb": emb_small, "at": at_core, "bt": bt_host}
        )

    nc = _build_graph(u_rows)
    res = run_bass_kernel_spmd(nc, in_maps, list(range(N_CORES)))
    out = np.concatenate(
        [np.asarray(res.results[i]["out"]).astype(np.float32) for i in range(N_CORES)],
        axis=0,
    )
    return out.reshape(B, S, DIM)
